# revision 27
# baseline (speedup 1.0000x reference)
"""Multi-head attention with q/v LoRA on 8 trn2 NeuronCores.

Reference computation (B=2, N=2048, C=1024, H=16, HD=64, R=16):
    qkv = x @ w_qkv + b_qkv                -> split per-head q, k, v
    q  += ((q @ a_q) @ b_q) * 2.0          (per head; same for v)
    out = softmax(q k^T / 8) v             (full N x N scores)
    y   = out @ w_proj + b_proj

Sharding: tensor-parallel over heads -- each of the 8 cores owns 2 heads
(128 of the 1024 qkv columns) for both batches; the attention output is
resharded over tokens with a 1 MB (bf16) AllToAll so each core computes
final proj rows for its 256 tokens per batch with the full w_proj.

v2 changes vs the fp32r baseline:
  * LoRA is folded into the qkv weights on the host:
      q + (q A) B s = q (I + A B s)  =>  W_q' = W_q (I + A B s),
      b_q' = b_q (I + A B s)   (exact; same for v).  No device LoRA.
  * all matmul operands are bf16 (x^T, w_qkv, q/k/v, P=exp(S), V_aug,
    w_proj, AllToAll payload); PSUM accumulation stays fp32.  This
    halves SBUF/HBM/collective traffic and sidesteps the fp32r power
    throttle that capped the PE at ~50% duty in the baseline.
  * softmax normalization: the PSUM ones-column gives column sums;
    reciprocal via the fast custom-DVE op on the [1,1024] sums row,
    PE ones-broadcast of the reciprocals, fused multiply into the bf16
    AllToAll staging tile.  The finish sequence of unit i is emitted
    inside unit i+1's score loop so the PE never stalls on it.
  * AllToAll results DMA straight into the proj input tile (no staging
    copy); v_aug transposes land in one PSUM tile moved by one strided
    DVE copy; ones column via one gpsimd memset.
  * tail: proj(batch 0) is emitted after the last AllToAll so the
    collective is covered by compute instead of exposing ~36 us.
"""

import sys

sys.path.insert(0, "/opt/trn_rl_repo")
sys.path.insert(0, "/root/.axon_site")

import numpy as np
import ml_dtypes

import concourse.bass as bass
import concourse.mybir as mybir
import concourse.tile as tile
from concourse.bass_utils import run_bass_kernel_spmd

f32 = mybir.dt.float32
f32r = mybir.dt.float32r
bf16 = mybir.dt.bfloat16
fp8 = mybir.dt.float8e4
AF = mybir.ActivationFunctionType

B, N, C = 2, 2048, 1024
H, HD, R = 16, 64, 16
LORA_SCALE = 32.0 / R
ATTN_SCALE = HD ** -0.5
NCORES = 8
HPC = H // NCORES          # heads per core = 2
PC = HPC * HD              # partition columns per core = 128
ROWS = B * N               # 4096 tokens
RC = 256                   # row-chunk size for qkv production
TPC = N // NCORES          # tokens per core per batch = 256


def _legalize_waits(nc, max_waits=1):
    """This walrus build accepts at most one sync-wait per instruction;
    Tile attaches several.  Move surplus waits onto same-engine NoOps
    inserted immediately before the instruction (identical semantics)."""
    counter = 0
    for fn in nc.m.functions:
        for bb in fn.blocks:
            insts = bb.instructions
            out = []
            changed = False
            for inst in insts:
                si = inst.sync_info
                if si is not None and si.on_wait and len(si.on_wait) > max_waits:
                    waits = list(si.on_wait)
                    for w in waits[:-max_waits]:
                        counter += 1
                        nop = mybir.InstNoOp(
                            name=f"I-wfix-{counter}",
                            engine=inst.engine,
                            sync_info=mybir.SyncInfo(on_wait=[w], on_update=[]),
                        )
                        nc.register_instruction(nop)
                        out.append(nop)
                    si.on_wait.clear()
                    si.on_wait.extend(waits[-max_waits:])
                    changed = True
                out.append(inst)
            if changed:
                insts[:] = out


def build_nc():
    nc = bass.Bass(num_devices=NCORES)

    xt_d = nc.dram_tensor("xt", [C, ROWS], bf16, kind="ExternalInput")
    wq_d = nc.dram_tensor("wq", [128, 1024], bf16, kind="ExternalInput")
    wk_d = nc.dram_tensor("wk", [128, 1024], bf16, kind="ExternalInput")
    wv_d = nc.dram_tensor("wv", [128, 1024], bf16, kind="ExternalInput")
    bq_d = nc.dram_tensor("bq", [128, 1], f32, kind="ExternalInput")
    bk_d = nc.dram_tensor("bk", [128, 1], f32, kind="ExternalInput")
    bv_d = nc.dram_tensor("bv", [128, 1], f32, kind="ExternalInput")
    wp_d = nc.dram_tensor("wp", [128, 8 * 1024], bf16, kind="ExternalInput")
    bp_d = nc.dram_tensor("bp", [128, 8], f32, kind="ExternalInput")
    eye64x2_d = nc.dram_tensor("eye64x2", [128, 64], bf16, kind="ExternalInput")
    out_d = nc.dram_tensor("out", [B, C, TPC], f32, kind="ExternalOutput")

    with nc.allow_low_precision(
        reason="bf16 matmul operands are intended; PSUM accumulation stays fp32"
    ), tile.TileContext(nc) as tc:
        with (
            tc.tile_pool(name="persist", bufs=1) as persist,
            tc.tile_pool(name="const", bufs=1) as const,
            tc.tile_pool(name="dram", bufs=1, space="DRAM") as dram,
            tc.tile_pool(name="xio", bufs=3) as xio_p,
            tc.tile_pool(name="work", bufs=2) as work_p,
            tc.tile_pool(name="ps", bufs=1, space="PSUM") as ps,
        ):
            # q/k/v bf16: fp8e4m3 q/k was tried for the scores matmul (lower
            # PE power -> less throttle) but its ~3.6% sigma quantization puts
            # the final error at ~4e-2, over the 2e-2 gate.
            qT = persist.tile([128, ROWS], bf16, tag="qT", name="qT")
            kT = persist.tile([128, ROWS], bf16, tag="kT", name="kT")
            vT = persist.tile([128, ROWS], bf16, tag="vT", name="vT")

            # prefetch the first x^T chunk's DMA ahead of the weight DMAs
            xstg00 = xio_p.tile([128, 8 * RC], bf16, tag="xstg", name="xs00")
            nc.sync.dma_start(
                out=xstg00[:].rearrange("p (a r) -> p a r", a=8),
                in_=xt_d[:, 0:RC].rearrange("(a p) r -> p a r", p=128),
            )

            def loaded(name, dram_t, shape, dt):
                t = const.tile(list(shape), dt, tag=name, name=name)
                nc.sync.dma_start(out=t[:], in_=dram_t[:])
                return t

            w_t = [
                loaded("wq_t", wq_d, (128, 1024), bf16),
                loaded("wk_t", wk_d, (128, 1024), bf16),
                loaded("wv_t", wv_d, (128, 1024), bf16),
            ]
            eye64x2 = loaded("eye64", eye64x2_d, (128, 64), bf16)
            bias_t = [
                loaded("bq", bq_d, (128, 1), f32),
                loaded("bk", bk_d, (128, 1), f32),
                loaded("bv", bv_d, (128, 1), f32),
            ]
            bp_t = loaded("bp", bp_d, (128, 8), f32)

            ones_s = const.tile([1, 64], f32, tag="ones_s", name="ones_s")
            nc.gpsimd.memset(ones_s[:], 1.0)
            ones_row = const.tile([1, 64], f32r, tag="ones_r", name="ones_r")
            nc.vector.tensor_copy(ones_row[:], ones_s[:])

            wp_t = const.tile([128, 8 * 1024], bf16, tag="wp_t", name="wp_t")

            qkvT = (qT, kT, vT)

            def emit_qkv_chunk(b, rci, xstg=None, act_ok=True):
                r0 = b * N + rci * RC
                if xstg is None:
                    xstg = xio_p.tile([128, 8 * RC], bf16, tag="xstg", name=f"xs{b}{rci}")
                    nc.sync.dma_start(
                        out=xstg[:].rearrange("p (a r) -> p a r", a=8),
                        in_=xt_d[:, r0 : r0 + RC].rearrange("(a p) r -> p a r", p=128),
                    )
                for m in range(3):
                    acc = ps.tile([128, RC], f32, tag="acc", bufs=2, name=f"ac{b}{rci}{m}")
                    for ci in range(8):
                        nc.tensor.matmul(
                            acc[:],
                            w_t[m][:, ci * 128 : (ci + 1) * 128],
                            xstg[:, ci * RC : (ci + 1) * RC],
                            start=(ci == 0),
                            stop=(ci == 7),
                        )
                    dst = qkvT[m][:, r0 : r0 + RC]
                    if m == 0 and act_ok:
                        nc.scalar.activation(dst, acc[:], AF.Identity, bias=bias_t[m][:])
                    else:
                        nc.vector.tensor_scalar_add(dst, acc[:], bias_t[m][:])

            def emit_vaug(b, hl):
                boff = b * N
                hs = slice(hl * HD, (hl + 1) * HD)
                vtr = ps.tile([128, 1024], bf16, tag="s", bufs=2, name=f"vt{b}{hl}")
                for kt in range(16):
                    ko = boff + kt * 128
                    nc.tensor.transpose(
                        vtr[:, kt * 64 : (kt + 1) * 64],
                        vT[hs, ko : ko + 128],
                        eye64x2[hs, :],
                    )
                v_aug = work_p.tile([128, 16 * 65], bf16, tag="vaug", name=f"va{b}{hl}")
                nc.gpsimd.memset(v_aug[:], 1.0)
                nc.vector.tensor_copy(
                    v_aug[:].rearrange("p (k c) -> p k c", c=65)[:, :, 0:64],
                    vtr[:].rearrange("p (k c) -> p k c", c=64),
                )
                return v_aug

            def act_recip_row(r_out, s_in, lg):
                """1/s = exp(-ln(s)) on the ACT engine.  Ln and Exp live in
                the same ACT table as the softmax exp, so this costs ~2us of
                ACT time with NO table reload -- unlike AF.Reciprocal (other
                table, 1.3us load each way) or the DVE reciprocal (6.5us,
                and it blocks the in-order DVE queue that also feeds the
                v_aug copy gating the next unit's PV matmuls)."""
                nc.scalar.activation(lg[:], s_in, AF.Ln)
                nc.scalar.activation(r_out, lg[:], AF.Exp, scale=-1.0)

            def emit_unit(b, hl, qh, v_aug, stage_to, fin=None, after_fin=None):
                """Scores -> exp -> PV for one (batch, head, q-half).

                The DVE part of the softmax normalization (PSUM evacuation +
                the ~6.5us reciprocal) is emitted at this unit's own end so it
                queues ahead of any interleaved qkv bias-adds on the DVE.  The
                PE part (ones-broadcast + multiply + a2a staging DMAs) is
                returned as a closure that the NEXT unit emits at kt==11,
                ~12us of PE work later, so it never stalls the PE.

                stage_to(nstb) emits the AllToAll staging DMAs; token shards
                are qh-interleaved (dst core c gets nstb[:, c*128:(c+1)*128]
                from each qh) so each (b, hl) collective's qh0 half is ready
                a unit early."""
                boff = b * N
                hs = slice(hl * HD, (hl + 1) * HD)
                qoff = boff + qh * 1024
                o_ps = ps.tile([65, 1024], f32, tag="o", bufs=1, name=f"o{b}{hl}{qh}")

                def emit_pv(p_tile, kt):
                    for qc in range(2):
                        nc.tensor.matmul(
                            o_ps[:, qc * 512 : (qc + 1) * 512],
                            v_aug[:, kt * 65 : kt * 65 + 65],
                            p_tile[:, qc * 512 : (qc + 1) * 512],
                            start=(kt == 0),
                            stop=(kt == 15),
                        )

                pending = None
                for kt in range(16):
                    ko = boff + kt * 128
                    s_ps = ps.tile([128, 1024], f32, tag="s", bufs=2, name=f"s{b}{hl}{qh}{kt}")
                    for qc in range(2):
                        nc.tensor.matmul(
                            s_ps[:, qc * 512 : (qc + 1) * 512],
                            kT[hs, ko : ko + 128],
                            qT[hs, qoff + qc * 512 : qoff + (qc + 1) * 512],
                            start=True,
                            stop=True,
                        )
                    p_sb = work_p.tile([128, 1024], bf16, tag="p", bufs=3, name=f"p{qh}{kt}")
                    nc.scalar.activation(p_sb[:], s_ps[:], AF.Exp, scale=ATTN_SCALE)
                    if pending is not None:
                        emit_pv(*pending)
                    pending = (p_sb, kt)
                    if kt == 11 and fin is not None:
                        fin()
                        if after_fin is not None:
                            after_fin()
                emit_pv(*pending)

                lg_sb = work_p.tile([1, 1024], f32, tag="lg", bufs=2, name=f"lg{b}{hl}{qh}")
                r_sb = work_p.tile([1, 1024], f32r, tag="r", bufs=2, name=f"rr{b}{hl}{qh}")
                nst = work_p.tile([64, 1024], f32r, tag="nst", bufs=2, name=f"n{hl}{qh}")
                nstb = work_p.tile([64, 1024], bf16, tag="nstb", bufs=2, name=f"nb{hl}{qh}")

                # O^T evacuation (DVE) in parallel with the sums reciprocal
                # (ACT, reading the PSUM sums row directly)
                nc.vector.tensor_copy(nst[:], o_ps[0:64, :])
                act_recip_row(r_sb[:], o_ps[64:65, :], lg_sb)

                def fin_pe():
                    for qc in range(2):
                        bc_ps = ps.tile([64, 512], f32, tag="acc", bufs=2, name=f"bc{qc}")
                        nc.tensor.matmul(
                            bc_ps[:],
                            ones_row[:],
                            r_sb[:, qc * 512 : (qc + 1) * 512],
                            start=True,
                            stop=True,
                        )
                        nc.vector.tensor_mul(
                            nstb[:, qc * 512 : (qc + 1) * 512],
                            nst[:, qc * 512 : (qc + 1) * 512],
                            bc_ps[:],
                        )
                    stage_to(nstb)

                return fin_pe

            # a2a buffers: [8, 64, 256] per (b, hl); collective time is
            # dominated by fixed overhead (~20us regardless of payload), so
            # fewer, larger collectives win.  Token shards stay
            # qh-interleaved so each unit stages its half independently.
            def new_ai(name, toks):
                return dram.tile([8, 64, toks], bf16, tag=name, name=name)

            ai = {(b, hl): new_ai(f"ai{b}{hl}", TPC)
                  for b, hl in ((0, 0), (0, 1), (1, 0), (1, 1))}

            def stage_full(b, hl, qh):
                def go(nstb):
                    nc.sync.dma_start(
                        out=ai[(b, hl)][:, :, qh * 128 : (qh + 1) * 128]
                        .rearrange("c p t -> p c t"),
                        in_=nstb[:].rearrange("p (c t) -> p c t", c=8),
                    )
                return go

            recv_tiles = {}

            def get_recv(b):
                if b not in recv_tiles:
                    recv_tiles[b] = work_p.tile(
                        [128, 8 * TPC], bf16, tag=f"rcr{b}", bufs=1, name=f"rr{b}"
                    )
                return recv_tiles[b]

            def emit_collective(a2a_in, toks, name):
                a2a_out = dram.tile([8, 64, toks], bf16, tag=name, name=name)
                nc.gpsimd.collective_compute(
                    "AllToAll",
                    mybir.AluOpType.bypass,
                    replica_groups=[list(range(NCORES))],
                    ins=[a2a_in[:].opt()],
                    outs=[a2a_out[:].opt()],
                )
                return a2a_out

            def fire_full(b, hl):
                def go():
                    a2a_out = emit_collective(ai[(b, hl)], TPC, f"ao{b}{hl}")
                    recv_r = get_recv(b)
                    nc.sync.dma_start(
                        out=recv_r[hl * 64 : (hl + 1) * 64, :]
                        .rearrange("p (c t) -> p c t", c=8),
                        in_=a2a_out[:].rearrange("c p t -> p c t"),
                    )
                return go



            def emit_proj_mt(b, recv_r, mts):
                for mt in mts:
                    y_ps = ps.tile([128, TPC], f32, tag="acc", bufs=2, name=f"y{b}{mt}")
                    for kc in range(8):
                        nc.tensor.matmul(
                            y_ps[:],
                            wp_t[:, kc * 1024 + mt * 128 : kc * 1024 + (mt + 1) * 128],
                            recv_r[:, kc * TPC : (kc + 1) * TPC],
                            start=(kc == 0),
                            stop=(kc == 7),
                        )
                    yst = work_p.tile([128, TPC], f32, tag="yst", bufs=3, name=f"ys{b}{mt}")
                    nc.vector.tensor_scalar_add(yst[:], y_ps[:], bp_t[:, mt : mt + 1])
                    nc.sync.dma_start(
                        out=out_d[b, mt * 128 : (mt + 1) * 128, :], in_=yst[:]
                    )

            # ---- emission schedule ----------------------------------------
            emit_qkv_chunk(0, 0, xstg=xstg00)
            for rci in range(1, 8):
                emit_qkv_chunk(0, rci)
            # full w_proj load: DMA engine is idle once x(b0) is in flight
            nc.sync.dma_start(out=wp_t[:], in_=wp_d[:])

            va = emit_vaug(0, 0)
            f = emit_unit(0, 0, 0, va, stage_full(0, 0, 0))
            emit_qkv_chunk(1, 0, act_ok=False)
            emit_qkv_chunk(1, 1, act_ok=False)
            f = emit_unit(0, 0, 1, va, stage_full(0, 0, 1), fin=f)
            emit_qkv_chunk(1, 2, act_ok=False)
            emit_qkv_chunk(1, 3, act_ok=False)
            va = emit_vaug(0, 1)
            f = emit_unit(0, 1, 0, va, stage_full(0, 1, 0), fin=f,
                          after_fin=fire_full(0, 0))
            emit_qkv_chunk(1, 4, act_ok=False)
            emit_qkv_chunk(1, 5, act_ok=False)
            f = emit_unit(0, 1, 1, va, stage_full(0, 1, 1), fin=f)
            emit_qkv_chunk(1, 6, act_ok=False)
            emit_qkv_chunk(1, 7, act_ok=False)
            va = emit_vaug(1, 0)
            f = emit_unit(1, 0, 0, va, stage_full(1, 0, 0), fin=f,
                          after_fin=fire_full(0, 1))
            f = emit_unit(1, 0, 1, va, stage_full(1, 0, 1), fin=f)
            # flush + fire (1,0) immediately: the collectives serialize on
            # the CC pipeline, so launching this one ~14us earlier unblocks
            # the tail-critical (1,1) collective by as much.  The vaug(1,1)
            # transposes fill the PE while the finish chain runs.
            va = emit_vaug(1, 1)
            f()
            fire_full(1, 0)()
            f = emit_unit(1, 1, 0, va, stage_full(1, 1, 0))
            f = emit_unit(1, 1, 1, va, stage_full(1, 1, 1), fin=f)
            # ---- tail: flush the last unit immediately (its ~5us finish
            # chain is filled with the first proj(0) tiles), launch the last
            # AllToAll, cover it with the rest of proj(0) plus a first
            # 64-row-contraction pass of proj(1) over head-pair 0 (whose
            # collective landed earlier); head-pair 1's contribution is added
            # in a second pass once the last collective lands.
            recv0 = get_recv(0)
            recv1 = get_recv(1)
            emit_proj_mt(0, recv0, range(0, 5))
            f()
            fire_full(1, 1)()
            emit_proj_mt(0, recv0, range(5, 8))
            y1_sb = work_p.tile([128, 8 * TPC], f32, tag="y1", bufs=1, name="y1")
            for mt in range(8):
                y_ps = ps.tile([128, TPC], f32, tag="acc", bufs=2, name=f"p1{mt}")
                for kc in range(8):
                    nc.tensor.matmul(
                        y_ps[:],
                        wp_t[0:64, kc * 1024 + mt * 128 : kc * 1024 + (mt + 1) * 128],
                        recv1[0:64, kc * TPC : (kc + 1) * TPC],
                        start=(kc == 0),
                        stop=(kc == 7),
                    )
                nc.vector.tensor_copy(y1_sb[:, mt * TPC : (mt + 1) * TPC], y_ps[:])
            for mt in range(8):
                y_ps = ps.tile([128, TPC], f32, tag="acc", bufs=2, name=f"p2{mt}")
                for kc in range(8):
                    nc.tensor.matmul(
                        y_ps[:],
                        wp_t[64:128, kc * 1024 + mt * 128 : kc * 1024 + (mt + 1) * 128],
                        recv1[64:128, kc * TPC : (kc + 1) * TPC],
                        start=(kc == 0),
                        stop=(kc == 7),
                    )
                yst = work_p.tile([128, TPC], f32, tag="yst", bufs=3, name=f"yf{mt}")
                nc.vector.scalar_tensor_tensor(
                    yst[:], y_ps[:], bp_t[:, mt : mt + 1],
                    y1_sb[:, mt * TPC : (mt + 1) * TPC],
                    mybir.AluOpType.add, mybir.AluOpType.add,
                )
                nc.sync.dma_start(
                    out=out_d[1, mt * 128 : (mt + 1) * 128, :], in_=yst[:]
                )

    _legalize_waits(nc)
    return nc


_NC_CACHE = None


def _get_nc():
    global _NC_CACHE
    if _NC_CACHE is None:
        _NC_CACHE = build_nc()
    return _NC_CACHE


def _make_in_maps(inputs):
    bf = ml_dtypes.bfloat16
    x = np.ascontiguousarray(np.asarray(inputs["x"], dtype=np.float32)).reshape(ROWS, C)
    xt = np.ascontiguousarray(x.T).astype(bf)   # [C, ROWS] bf16
    w_qkv = np.asarray(inputs["w_qkv"], dtype=np.float64)
    b_qkv = np.asarray(inputs["b_qkv"], dtype=np.float64)
    a_q = np.asarray(inputs["a_q"], dtype=np.float64)
    b_q = np.asarray(inputs["b_q"], dtype=np.float64)
    a_v = np.asarray(inputs["a_v"], dtype=np.float64)
    b_v = np.asarray(inputs["b_v"], dtype=np.float64)
    w_proj = np.asarray(inputs["w_proj"], dtype=np.float32)
    b_proj = np.asarray(inputs["b_proj"], dtype=np.float32)

    # fold per-head LoRA into the q/v projections:  q' = q (I + A B s)
    m_q = np.eye(HD) + a_q @ b_q * LORA_SCALE          # [64, 64]
    m_v = np.eye(HD) + a_v @ b_v * LORA_SCALE
    wq = np.ascontiguousarray(
        (w_qkv[:, 0 * C : 1 * C].reshape(C, H, HD) @ m_q).reshape(C, C)
    )
    wk = w_qkv[:, 1 * C : 2 * C]
    wv = np.ascontiguousarray(
        (w_qkv[:, 2 * C : 3 * C].reshape(C, H, HD) @ m_v).reshape(C, C)
    )
    bq = (b_qkv[0 * C : 1 * C].reshape(H, HD) @ m_q).reshape(C).astype(np.float32)
    bk = b_qkv[1 * C : 2 * C].astype(np.float32)
    bv = (b_qkv[2 * C : 3 * C].reshape(H, HD) @ m_v).reshape(C).astype(np.float32)

    eye64x2 = np.vstack([np.eye(64)] * 2).astype(bf)

    def warr(w):                              # [1024, n] -> [128, 8*n] chunk-major
        n = w.shape[1]
        return np.ascontiguousarray(
            w.reshape(8, 128, n).transpose(1, 0, 2).reshape(128, 8 * n)
        ).astype(bf)

    wp_full = warr(w_proj)                    # [128, 8*1024] bf16
    bp = np.ascontiguousarray(b_proj.reshape(8, 128).T)

    in_maps = []
    for c in range(NCORES):
        sl = slice(c * PC, (c + 1) * PC)
        in_maps.append(
            {
                "xt": xt,
                "wq": warr(wq[:, sl]),
                "wk": warr(np.ascontiguousarray(wk[:, sl])),
                "wv": warr(wv[:, sl]),
                "bq": np.ascontiguousarray(bq[sl].reshape(128, 1)),
                "bk": np.ascontiguousarray(bk[sl].reshape(128, 1)),
                "bv": np.ascontiguousarray(bv[sl].reshape(128, 1)),
                "wp": wp_full,
                "bp": bp,
                "eye64x2": eye64x2,
            }
        )
    return in_maps


# qh-interleaved token shard: core c's 256 output columns per batch are
# tokens [c*128, (c+1)*128) from the first q-half and 1024 + the same from
# the second q-half (see stage_full/stage_half in build_nc)
_TOK_IDX = np.concatenate(
    [np.r_[c * 128 : (c + 1) * 128, 1024 + c * 128 : 1024 + (c + 1) * 128]
     for c in range(NCORES)]
)


def run_sharded(inputs, trace=False, **kw):
    nc = _get_nc()
    in_maps = _make_in_maps(inputs)
    res = run_bass_kernel_spmd(nc, in_maps, list(range(NCORES)), trace=trace, **kw)
    # results[c]["out"]: [B, C, TPC] -- core c's token shard of final y^T
    yT = np.concatenate([res.results[c]["out"] for c in range(NCORES)], axis=2)
    out = np.empty((B, N, C), dtype=np.float32)
    out[:, _TOK_IDX, :] = yT.transpose(0, 2, 1)
    return out, res


def kernel(**inputs) -> np.ndarray:
    out, _ = run_sharded(inputs, trace=False)
    return out
